# revision 2
# baseline (speedup 1.0000x reference)
"""MoE-routed 3x3 conv (MixedLayerWithArc) on 8 TRN2 NeuronCores.

Reference semantics: out[i] = conv3x3(x[i], W[sample_arc[i]], b[sample_arc[i]]).

Strategy — 1D-Winograd(W) x direct(H) hybrid, transforms on the host:
  * The 2D F(4,3)xF(4,3) kernel was DMA-bound: 28.3 MB/core (xt 9.4 + wt 9.4
    + out 9.4) over ~354 GB/s = 80 us of traffic for only 31 us of PE. fp8
    can't fix it: Winograd-domain quantization is amplified ~12x (fp8e3
    weights alone measured 14.8% rel err vs the 2e-2 budget).
  * This version trades PE for bytes: 1D F(8,3) along W (4 tiles of 8 output
    cols, alpha=10 positions, half-integer points 0,+-1,+-2,+-1/2,+-3/2 for
    stability: 7.5e-3 end-to-end in f16), H stays DIRECT — the 3 vertical
    taps accumulate in PSUM from the same SBUF data at row offsets dy, so
    they cost no extra traffic.
  * Per-core traffic drops to 18.4 MB (V1 5.6 + U1 7.9 + M1 5.2) ~= 52 us,
    PE rises to 123k cycles ~= 51 us — balanced at the ridge.
  * Routing resolved on the HOST (same slot packing as before): samples are
    packed so every core's 8 samples group into K uniform slots; slot j's
    samples occupy a contiguous column range of every GEMM.
  * V1 is stored [P, NU, S, ROWS*NTX]; the (rows dy:dy+32 x all tx) block a
    matmul needs is 128 contiguous elements at offset dy*NTX, so every rhs
    is a plain 2D slice. Padded zero rows (0 and 33) ship as zeros.

Per-core tensors (m=8):
  xt  [2, 128, 10, 8, 136] f16     (ci_t, ci, nu, samp, row*4+tx)
  wt  [K, 2, 128, 10, 3, 2, 128] f16 (slot, ci_t, ci, nu, dy, co_t, co)
  out [2, 10, 128, 1024] f16       (co_t, nu, co, samp*128 + h*4 + tx)
"""
import os

import numpy as np

B, C, H, W_ = 64, 256, 32, 32
NB = 4                     # branches
NCORES = 8
SPC = B // NCORES          # samples per core
P = 128                    # partition tile
CT = C // P                # channel tiles (2)
DY = 3                     # direct vertical taps
M_OUT = 8                  # 1D winograd output tile (cols)
ALPHA = M_OUT + 2          # input tile / positions
NU = ALPHA
NTX = W_ // M_OUT          # winograd tiles per row
ROWS = H + 2               # padded rows shipped (0 and H+1 are zeros)
RB = ROWS * NTX            # per-sample row-block length (136)
CPS = H * NTX              # output cols per sample (128)
COLS = SPC * CPS           # total GEMM cols (1024)
CHUNK = 512                # PSUM bank (f32 words per partition)
SAMP_PER_CHUNK = CHUNK // CPS
NCHUNK = COLS // CHUNK
WARMUP = 24
# in-DMA chunking over the nu axis; small early chunks -> early PE start and
# no PE stalls (which would also reset the p-state ramp).
# Chunks are triple-buffered (pool reuse) so the DMA rings only ever hold
# ~3 chunks of in-descriptors: out-DMAs interleave instead of queueing
# behind the whole in-stream, and chunks complete in need-order.
NU_CHUNKS = [(0, 1), (1, 2), (2, 3), (3, 5), (5, 7), (7, 9), (9, 10)]
INBUFS = 3

# --- 1D F(8,3) Cook-Toom transforms, points {0,+-1,+-2,+-1/2,+-3/2,inf} ---
# (generated exactly via fractions; hardcoded here)
_PTS = [0.0, 1.0, -1.0, 2.0, -2.0, 0.5, -0.5, 1.5, -1.5]


def _cook_toom(points, m, r):
    from fractions import Fraction
    a = m + r - 1
    pts = [Fraction(p) for p in points]
    assert len(pts) == a - 1

    def poly_mul(p, q):
        out = [Fraction(0)] * (len(p) + len(q) - 1)
        for i, pi in enumerate(p):
            for j, qj in enumerate(q):
                out[i + j] += pi * qj
        return out

    def prod_poly(excl):
        p = [Fraction(1)]
        for k, pk in enumerate(pts):
            if k == excl:
                continue
            p = poly_mul(p, [-pk, Fraction(1)])
        return p

    Ni = [Fraction(1)] * (a - 1)
    for i in range(a - 1):
        for k in range(a - 1):
            if k != i:
                Ni[i] *= (pts[i] - pts[k])

    G = np.zeros((a, r))
    for i in range(a - 1):
        for j in range(r):
            G[i, j] = float(pts[i] ** j / Ni[i])
    G[a - 1, r - 1] = 1.0

    AT = np.zeros((m, a))
    for j in range(a - 1):
        for i in range(m):
            AT[i, j] = float(pts[j] ** i)
    AT[m - 1, a - 1] = 1.0

    BT = np.zeros((a, a))
    for i in range(a - 1):
        mp = prod_poly(i)
        for j, cj in enumerate(mp):
            BT[i, j] = float(cj)
    mfull = prod_poly(None)
    for j, cj in enumerate(mfull):
        BT[a - 1, j] = float(cj)
    return (AT.astype(np.float32), G.astype(np.float32),
            BT.astype(np.float32))


AT1, G1, BT1 = _cook_toom(_PTS, M_OUT, 3)

# slot-size patterns in cost order (fewer slots = less weight DMA)
PATTERNS = [
    (8,), (5, 3), (6, 2), (7, 1), (4, 4),
    (4, 3, 1), (3, 3, 2), (4, 2, 2), (5, 2, 1), (6, 1, 1),
    (2, 2, 2, 2), (3, 2, 2, 1), (3, 3, 1, 1), (4, 2, 1, 1), (5, 1, 1, 1),
    (2, 2, 2, 1, 1), (4, 1, 1, 1, 1), (2, 2, 1, 1, 1, 1),
    (2, 1, 1, 1, 1, 1, 1), (1,) * 8,
]

TRACE = False
TRACE_DIR = None
LAST_RESULTS = None

_prog_cache = {}


def _pattern_assign(counts, sizes):
    K = len(sizes)
    caps = [NCORES] * K
    m = []

    def branch_vecs(j, rem):
        if j == K:
            if rem == 0:
                yield []
            return
        for v in range(min(caps[j], rem // sizes[j]), -1, -1):
            for rest in branch_vecs(j + 1, rem - v * sizes[j]):
                yield [v] + rest

    def dfs(b):
        if b == NB:
            return all(c == 0 for c in caps)
        for v in branch_vecs(0, counts[b]):
            for j in range(K):
                caps[j] -= v[j]
            m.append(v)
            if dfs(b + 1):
                return True
            m.pop()
            for j in range(K):
                caps[j] += v[j]
        return False

    return m if dfs(0) else None


def _plan_routing(arc):
    counts = np.bincount(arc, minlength=NB).tolist()
    for sizes in PATTERNS:
        m = _pattern_assign(counts, sizes)
        if m is not None:
            break
    assert m is not None
    K = len(sizes)
    slot_branches = np.empty((NCORES, K), np.int64)
    for j in range(K):
        lst = []
        for br in range(NB):
            lst += [br] * m[br][j]
        slot_branches[:, j] = lst
    pools = [list(np.nonzero(arc == br)[0]) for br in range(NB)]
    perm = np.empty(B, np.int64)
    i = 0
    for c in range(NCORES):
        for j, sz in enumerate(sizes):
            br = slot_branches[c, j]
            for _ in range(sz):
                perm[i] = pools[br].pop()
                i += 1
    return sizes, slot_branches, perm


def _segments(sizes):
    """Per chunk: list of (slot j, s0, s1) sample sub-ranges."""
    sb = [0]
    for s in sizes:
        sb.append(sb[-1] + s)
    out = []
    for c in range(NCHUNK):
        cs, ce = c * SAMP_PER_CHUNK, (c + 1) * SAMP_PER_CHUNK
        segs = []
        for j in range(len(sizes)):
            s0, s1 = max(sb[j], cs), min(sb[j + 1], ce)
            if s0 < s1:
                segs.append((j, s0, s1))
        out.append(segs)
    return out


def _build_program(sizes):
    import concourse.tile as tile
    from concourse import bacc, mybir

    K = len(sizes)
    chunk_segs = _segments(sizes)

    nc = bacc.Bacc("TRN2", target_bir_lowering=False, debug=False,
                   num_devices=NCORES)
    f32 = mybir.dt.float32
    f16 = mybir.dt.float16

    xt_d = nc.dram_tensor("xt", [CT, P, NU, SPC, RB], f16,
                          kind="ExternalInput").ap()
    wt_d = nc.dram_tensor("wt", [K, CT, P, NU, DY, CT, P], f16,
                          kind="ExternalInput").ap()
    out_d = nc.dram_tensor("out", [CT, NU, P, COLS], f16,
                           kind="ExternalOutput").ap()

    MAXW = max(hi - lo for lo, hi in NU_CHUNKS)
    with tile.TileContext(nc) as tc:
        with tc.tile_pool(name="spool", bufs=1) as spool, \
             tc.tile_pool(name="xpool", bufs=INBUFS) as xpool, \
             tc.tile_pool(name="wpool", bufs=INBUFS) as wpool, \
             tc.tile_pool(name="opool", bufs=6) as opool, \
             tc.tile_pool(name="psum", bufs=8, space="PSUM") as psum_pool:

            # PE warmup: dummy matmuls during the initial DMA fill so the
            # p-state clock ramps before the first real matmul.
            scratch = spool.tile([P, P], f16, name="scratch", tag="scr")
            nc.gpsimd.memset(scratch[:], 0.0)
            ps_warm = psum_pool.tile([P, CHUNK], f32, name="ps_warm", tag="ps")
            for _ in range(WARMUP):
                nc.tensor.matmul(ps_warm[:, :P], scratch[:], scratch[:],
                                 start=True, stop=True, skip_group_check=True)

            # per-chunk double-buffered in-tiles; DMA for chunk k can only
            # issue after chunk k-2's consumers are done (pool WAR dep), so
            # the rings stay shallow and outs interleave.
            xcs, wcs = [], []
            for ck, (lo, hi) in enumerate(NU_CHUNKS):
                w = hi - lo
                sl = slice(lo, hi)
                xc = [xpool.tile([P, MAXW, SPC, RB], f16,
                                 name=f"xc{ck}_{ci}", tag=f"x{ci}")
                      for ci in range(CT)]
                wc = [[wpool.tile([P, MAXW, DY, CT, P], f16,
                                  name=f"wc{ck}_{j}_{ci}", tag=f"w{j}_{ci}")
                       for ci in range(CT)] for j in range(K)]
                # queue balance: ci=0 xt + ci=1 wt on sync; mirror on scalar.
                # Order matches first-use: xt0 and wt[0][0] lead so the first
                # matmul's operands arrive first.
                nc.sync.dma_start(xc[0][:, :w], xt_d[0][:, sl])
                nc.scalar.dma_start(wc[0][0][:, :w], wt_d[0, 0][:, sl])
                for j in range(K):
                    nc.sync.dma_start(wc[j][1][:, :w], wt_d[j, 1][:, sl])
                nc.scalar.dma_start(xc[1][:, :w], xt_d[1][:, sl])
                for j in range(1, K):
                    nc.scalar.dma_start(wc[j][0][:, :w], wt_d[j, 0][:, sl])
                xcs.append(xc)
                wcs.append(wc)

            for ck, (lo, hi) in enumerate(NU_CHUNKS):
                xc, wc = xcs[ck], wcs[ck]
                for ni in range(hi - lo):
                    nu = lo + ni
                    for co_t in range(CT):
                        ot = opool.tile([P, COLS], f16, name=f"ot{nu}_{co_t}",
                                        tag="ot")
                        # NOTE: accumulation groups must stay CONTIGUOUS per
                        # PSUM region — interleaving groups across banks (to
                        # chain same-stationary matmuls) silently dropped one
                        # accumulation term on HW.
                        for c in range(NCHUNK):
                            ps = psum_pool.tile([P, CHUNK], f32,
                                                name=f"ps{nu}_{co_t}_{c}",
                                                tag="ps")
                            for (j, s0, s1) in chunk_segs[c]:
                                pc0 = (s0 - c * SAMP_PER_CHUNK) * CPS
                                pc1 = (s1 - c * SAMP_PER_CHUNK) * CPS
                                for dy in range(DY):
                                    for ci in range(CT):
                                        nc.tensor.matmul(
                                            ps[:, pc0:pc1],
                                            lhsT=wc[j][ci][:, ni, dy,
                                                           co_t, :],
                                            rhs=xc[ci][:, ni, s0:s1,
                                                       dy * NTX:
                                                       dy * NTX + CPS],
                                            start=(dy == 0 and ci == 0),
                                            stop=(dy == DY - 1 and
                                                  ci == CT - 1))
                            # evictions alternate DVE/ACT
                            if ((nu * CT + co_t) * NCHUNK + c) % 2 == 0:
                                nc.vector.tensor_copy(
                                    ot[:, c * CHUNK:(c + 1) * CHUNK], ps[:])
                            else:
                                nc.scalar.copy(
                                    ot[:, c * CHUNK:(c + 1) * CHUNK], ps[:])
                        # out block on the SWDGE queue
                        nc.gpsimd.dma_start(out_d[co_t, nu], ot[:])
    nc.compile()
    return nc


def kernel(x, sample_arc, W, b):
    global LAST_RESULTS

    x = np.asarray(x, dtype=np.float32)
    arc = np.asarray(sample_arc).astype(np.int64)
    W = np.asarray(W, dtype=np.float32)
    b = np.asarray(b, dtype=np.float32)

    sizes, slot_branches, perm = _plan_routing(arc)

    # ---- host input transform: V1 = (1D B^T) over alpha-col windows ----
    xp = np.zeros((B, C, ROWS, W_ + 2), np.float32)
    xp[:, :, 1:1 + H, 1:1 + W_] = x
    wins = np.lib.stride_tricks.sliding_window_view(
        xp, (ALPHA,), axis=(3,))[:, :, :, ::M_OUT]    # [B,C,ROWS,NTX,ALPHA]
    V1 = np.einsum('ui,bcrti->bcrtu', BT1, wins)      # [B,C,ROWS,NTX,NU]
    Vp = V1[perm].reshape(NCORES, SPC, CT, P, ROWS, NTX, NU)
    xt = np.ascontiguousarray(
        Vp.transpose(0, 2, 3, 6, 1, 4, 5).reshape(
            NCORES, CT, P, NU, SPC, RB)).astype(np.float16)

    # ---- host weight transform: U1 = G w (along kx) ----
    U1 = np.einsum('ui,boadi->boadu', G1, W)          # [NB,CO,CI,DY,NU]
    U1l = np.ascontiguousarray(
        U1.reshape(NB, CT, P, CT, P, DY, NU)
        .transpose(0, 3, 4, 6, 5, 1, 2)
        .reshape(NB, CT, P, NU, DY, CT, P)).astype(np.float16)
    wt = U1l[slot_branches]                 # [NCORES,K,CT,P,NU,DY,CT,P]

    in_maps = [{"xt": xt[c], "wt": np.ascontiguousarray(wt[c])}
               for c in range(NCORES)]

    if os.environ.get("KERNEL_EMULATE") == "1":
        results = _emulate(in_maps, sizes)
        LAST_RESULTS = None
    else:
        from concourse.bass_utils import run_bass_kernel_spmd
        key = sizes
        nc = _prog_cache.get(key)
        if nc is None:
            nc = _prog_cache[key] = _build_program(sizes)
        res = run_bass_kernel_spmd(nc, in_maps, core_ids=list(range(NCORES)),
                                   trace=TRACE, tmpdir=TRACE_DIR)
        LAST_RESULTS = res
        results = res.results

    # ---- host output transform: Y = M1 A (over nu), + bias, un-permute ----
    M = np.stack([np.asarray(results[c]["out"]) for c in range(NCORES)])
    M32 = M.astype(np.float32).reshape(NCORES, CT, NU, P, SPC, H, NTX)
    # Y[k, s, co_t, co, h, tx, m] = sum_u AT1[m,u] M32[k,co_t,u,co,s,h,tx]
    Y = np.einsum('mu,kcupshx->kscphxm', AT1, M32, optimize=True)
    Y = np.ascontiguousarray(Y).reshape(B, C, H, W_)
    Y += b[arc[perm]][:, :, None, None]
    out = np.empty_like(Y)
    out[perm] = Y
    return out


def _emulate(in_maps, sizes):
    """Numpy stand-in for the device program (layout/packing validation)."""
    results = []
    for im in in_maps:
        xt = im["xt"].astype(np.float32)   # [CT,P,NU,SPC,RB]
        wt = im["wt"].astype(np.float32)   # [K,CT,P,NU,DY,CT,P]
        sb = [0]
        for s in sizes:
            sb.append(sb[-1] + s)
        M = np.zeros((CT, NU, P, SPC, H, NTX), np.float32)
        xr = xt.reshape(CT, P, NU, SPC, ROWS, NTX)
        for j in range(len(sizes)):
            s0, s1 = sb[j], sb[j + 1]
            for dy in range(DY):
                # [CT,P,NU,samp,H,NTX] x [CT,P,NU,DY,CT,P] -> [CT,NU,P,...]
                M[:, :, :, s0:s1] += np.einsum(
                    'ipushx,ipuoq->ouqshx',
                    xr[:, :, :, s0:s1, dy:dy + H], wt[j, :, :, :, dy],
                    optimize=True)
        results.append({"out": np.ascontiguousarray(
            M.reshape(CT, NU, P, COLS)).astype(np.float16)})
    return results
